# revision 23
# baseline (speedup 1.0000x reference)
"""Trainium2 Bass kernel for nn_Decoder: 2-layer LSTM decoder + log-softmax NLL.

Strategy: pure 8-way data parallel over batch (B=256 -> 32 rows/core), zero
collectives. Per core:
  pre:     batched precompute of the non-recurrent layer-0 gate contribution
           pre[t,b,:] = e @ W0e + z @ W0z + bg0 (full-width matmuls, PSUM ->
           DRAM scratch), re-injected per step with one identity matmul
  phase 0: transformh0 (z -> initial h/c per layer) on device
  phase 1: 39 recurrent LSTM steps; weights streamed through the PE as the
           moving operand (stationary = transposed activations, M=32);
           per-gate PSUM slices + per-gate activations for early release;
           layer-1 elementwise tail deferred past the next step's layer-0
           matmuls (software-pipelined emission)
  phase 2: vocab projection batched over (t, b) -> logsumexp via ACT exp with
           fused accum_out; target logit via elementwise mul + ones-matmul
           partition reduction against host-pregathered Wout rows.
Host does: embedding gather, weight transposes/reshapes, final sum over t.
LSTM matmul operands bf16 (fp32 PSUM accumulate); vocab matmuls float32r.
"""

import numpy as np
import ml_dtypes

import concourse.tile as tile
import concourse.mybir as mybir
from concourse import bacc
from concourse import bass_utils

B, T, V, D, Z = 256, 40, 5000, 512, 128
NC = 8
BL = B // NC            # 32 batch rows per core
NT = T - 1              # 39 recurrent steps / vocab rows per b
COLS = NT * BL          # 1248 (t, b) columns per core
G = 4 * D               # 2048 gate width
NTILE = (COLS + 127) // 128   # 10 vocab tiles (last has 96 cols)

bf16 = mybir.dt.bfloat16
f32 = mybir.dt.float32
f32r = mybir.dt.float32r
AF = mybir.ActivationFunctionType

# gate order in the fused weight layout: i, f, o, cn
GI, GF, GO, GC = 0, 1, 2, 3

_CACHE = {}


def _build():
    nc = bacc.Bacc("TRN2", target_bir_lowering=False, debug=False)

    def din(name, shape, dt):
        return nc.dram_tensor(name, shape, dt, kind="ExternalInput").ap()

    zT_d = din("zT", [128, BL], bf16)
    zrepb_d = din("zrepb", [128, 128], bf16)
    eT_d = din("eT", [128, 4 * T * BL], bf16)
    w0h_d = din("w0h", [128, 4 * G], bf16)
    w0e_d = din("w0e", [128, 4 * G], bf16)
    w0z_d = din("w0z", [128, G], bf16)
    bg0_d = din("bg0r", [1, G], bf16)
    w1_d = din("w1", [128, 8 * G], bf16)
    bg1_d = din("bg1r", [1, G], bf16)
    tw1_d = din("tw1T", [128, 2 * G], bf16)
    tb1_d = din("tb1r", [1, 2 * G], bf16)
    tw2_d = din("tw2T", [128, 2 * 16 * 1024], bf16)
    tb2_d = din("tb2r", [1, 2 * 1024], bf16)
    wout_d = din("woutT", [128, 5 * V], bf16)
    bout_d = din("boutr", [1, V], bf16)
    wta_d = din("wtaT", [128, 5 * COLS], f32r)
    id32_d = din("id32", [32, 32], f32)
    id32b_d = din("id32b", [32, 32], bf16)
    selc_d = din("selc", [128, 128], bf16)
    ones32_d = din("ones32", [1, BL], bf16)
    ones128b_d = din("ones128b", [1, 128], bf16)
    onescol_d = din("onescol", [128, 2], f32r)
    out_d = nc.dram_tensor("out_lp", [COLS, 1], f32, kind="ExternalOutput").ap()

    with tile.TileContext(nc) as tc:
        from contextlib import ExitStack
        with ExitStack() as ctx:
            const = ctx.enter_context(tc.tile_pool(name="const", bufs=1))
            state = ctx.enter_context(tc.tile_pool(name="state", bufs=1))
            state2 = ctx.enter_context(tc.tile_pool(name="state2", bufs=2))

            def cload(shape, dt, dram, tag):
                t = const.tile(shape, dt, tag=tag)
                nc.sync.dma_start(t[:], dram[:])
                return t

            zT = cload([128, BL], bf16, zT_d, "c_zT")
            zrepb = cload([128, 128], bf16, zrepb_d, "c_zrepb")
            id32 = cload([32, 32], f32, id32_d, "c_id32")
            id32b = cload([32, 32], bf16, id32b_d, "c_id32b")
            selc = cload([128, 128], bf16, selc_d, "c_selc")
            ones32 = cload([1, BL], bf16, ones32_d, "c_ones32")
            ones128b = cload([1, 128], bf16, ones128b_d, "c_ones128b")
            onescol = cload([128, 2], f32r, onescol_d, "c_onescol")
            bg0 = cload([1, G], bf16, bg0_d, "c_bg0")
            bg1 = cload([1, G], bf16, bg1_d, "c_bg1")

            HT = state.tile([128, 4 * COLS], bf16)
            preS = state.tile([128, NTILE * G], bf16, tag="preS")
            lses = state.tile([128, 16], f32, tag="lses")

            # recurrent-loop weights: pool reserved early so the DMAs can
            # stream during phase 0 / precompute without address conflicts
            p1w_cm = tc.tile_pool(name="p1w", bufs=1)
            p1w = p1w_cm.__enter__()

            # phase-0 weights (tw2 per-layer shared slot)
            p0w_cm = tc.tile_pool(name="p0w", bufs=1)
            p0w = p0w_cm.__enter__()
            tw1 = p0w.tile([128, 2 * G], bf16, tag="tw1")
            nc.sync.dma_start(tw1[:], tw1_d[:])
            # precompute inputs next in DMA priority order
            ppw_cm = tc.tile_pool(name="ppw", bufs=1)
            ppw = ppw_cm.__enter__()
            w0e = ppw.tile([128, 4 * G], bf16)
            nc.sync.dma_start(w0e[:], w0e_d[:])
            w0z = ppw.tile([128, G], bf16)
            nc.sync.dma_start(w0z[:], w0z_d[:])
            eT = ppw.tile([128, 4 * T * BL], bf16)
            nc.sync.dma_start(eT[:], eT_d[:])
            tw2a = p0w.tile([128, 16 * 1024], bf16, tag="tw2")
            nc.sync.dma_start(tw2a[:], tw2_d[:, 0:16384])
            w0h = p1w.tile([128, 4 * G], bf16)
            nc.sync.dma_start(w0h[:], w0h_d[:])
            w1 = p1w.tile([128, 8 * G], bf16)

            # ---------------- phase 0: transformh0 -------------------------
            # emitted before the precompute so the precompute matmuls fill the
            # PE gaps left by phase 0's transpose/activation chains
            c_prev = [None, None]
            hT_init = [None, None]
            with tc.tile_pool(name="p0s", bufs=1) as p0s, \
                 tc.tile_pool(name="p0pa", bufs=1, space="PSUM") as p0pa, \
                 tc.tile_pool(name="p0tr", bufs=2, space="PSUM") as p0tr, \
                 tc.tile_pool(name="ppp", bufs=2, space="PSUM") as ppp:
                def phase0_layer(layer):
                    if layer == 0:
                        tw2 = tw2a
                    else:
                        tw2 = p0w.tile([128, 16 * 1024], bf16, tag="tw2")
                        nc.sync.dma_start(
                            tw2[:], tw2_d[:, 16384:32768])
                    tb1 = p0w.tile([1, G], bf16, tag="tb1")
                    nc.sync.dma_start(tb1[:], tb1_d[0:1, layer * G:(layer + 1) * G])
                    tb2 = p0w.tile([1, 1024], bf16, tag="tb2")
                    nc.sync.dma_start(
                        tb2[:], tb2_d[0:1, layer * 1024:(layer + 1) * 1024])
                    pa = p0pa.tile([BL, G], f32, tag="pa")
                    for s in range(4):
                        ns = slice(512 * s, 512 * s + 512)
                        nc.tensor.matmul(pa[:, ns], zT[:, :],
                                         tw1[:, layer * G + 512 * s:
                                             layer * G + 512 * s + 512],
                                         start=True, stop=False)
                        nc.tensor.matmul(pa[:, ns], ones32[0:1, :],
                                         tb1[0:1, 512 * s:512 * s + 512],
                                         start=False, stop=True)
                    u = p0s.tile([BL, G], bf16, tag="u")
                    nc.scalar.activation(u[:], pa[:], AF.Relu)
                    uT = p0s.tile([128, 16 * 32], bf16, tag="uT")
                    for c in range(16):
                        pt = p0tr.tile([128, 32], bf16, tag="tr")
                        nc.tensor.transpose(pt[:], u[:, 128 * c:128 * c + 128],
                                            id32b[:])
                        nc.vector.tensor_copy(uT[:, 32 * c:32 * c + 32], pt[:])
                    pb = p0pa.tile([BL, G], f32, tag="pa")
                    for s in range(2):
                        ns = slice(512 * s, 512 * s + 512)
                        for c in range(16):
                            nc.tensor.matmul(
                                pb[:, ns], uT[:, 32 * c:32 * c + 32],
                                tw2[:, c * 1024 + 512 * s:
                                    c * 1024 + 512 * s + 512],
                                start=(c == 0), stop=False)
                        nc.tensor.matmul(pb[:, ns], ones32[0:1, :],
                                         tb2[0:1, 512 * s:512 * s + 512],
                                         start=False, stop=True)
                    v = state.tile([BL, 1024], f32, tag=f"v{layer}")
                    nc.scalar.activation(v[:], pb[:, 0:1024], AF.Tanh)
                    hT = state.tile([128, 128], bf16, tag=f"hTi{layer}")
                    for c in range(4):
                        pt = p0tr.tile([128, 32], f32, tag="tr")
                        nc.tensor.transpose(pt[:], v[:, 128 * c:128 * c + 128],
                                            id32[:])
                        nc.vector.tensor_copy(hT[:, 32 * c:32 * c + 32], pt[:])
                    hT_init[layer] = hT
                    c_prev[layer] = v[:, 512:1024]

                # ------- precompute pre[t,b,:] = eW0e + zW0z + bg0 ---------
                def pre_tile(j):
                    for q in range(4):
                        go = 512 * q
                        pp = ppp.tile([128, 512], f32, tag="pp")
                        for c in range(4):
                            nc.tensor.matmul(
                                pp[:, :],
                                eT[:, c * T * BL + 128 * j:
                                   c * T * BL + 128 * j + 128],
                                w0e[:, c * G + go:c * G + go + 512],
                                start=(c == 0), stop=False)
                        nc.tensor.matmul(pp[:, :], zrepb[:, :],
                                         w0z[:, go:go + 512],
                                         start=False, stop=False)
                        nc.tensor.matmul(pp[:, :], ones128b[0:1, :],
                                         bg0[0:1, go:go + 512],
                                         start=False, stop=True)
                        nc.scalar.copy(preS[:, j * G + go:j * G + go + 512],
                                       pp[:, :])

                phase0_layer(0)
                for j in range(8):
                    pre_tile(j)
                phase0_layer(1)
                nc.sync.dma_start(w1[:], w1_d[:])
                for j in range(8, NTILE):
                    pre_tile(j)

            ppw_cm.__exit__(None, None, None)
            p0w_cm.__exit__(None, None, None)

            # phase-2 vocab weights: load during phase 1 (DMA idle there)
            p2w_cm = tc.tile_pool(name="p2w", bufs=1)
            p2w = p2w_cm.__enter__()
            wout = p2w.tile([128, 5 * V], bf16)
            nc.gpsimd.dma_start(wout[:], wout_d[:])
            bout = p2w.tile([1, V], bf16)
            nc.gpsimd.dma_start(bout[:], bout_d[:])

            # ---------------- phase 1: 39 LSTM steps -----------------------
            # vocab logits tiles are interleaved into the loop as PE filler
            groups = [(0, 1024), (1024, 1024), (2048, 1024),
                      (3072, 1024), (4096, 904)]
            with tc.tile_pool(name="p1g", bufs=4, space="PSUM") as p1g, \
                 tc.tile_pool(name="p1tr", bufs=2, space="PSUM") as p1tr, \
                 tc.tile_pool(name="p1e", bufs=2) as p1e, \
                 tc.tile_pool(name="p2s", bufs=2) as p2s, \
                 tc.tile_pool(name="p2pl", bufs=1, space="PSUM") as p2pl:
                h0T, h1T = hT_init
                c0, c1 = c_prev
                pend = None   # deferred layer-1 tail of the previous step

                def transpose4(src, dst):
                    for c in range(4):
                        pt = p1tr.tile([128, 32], bf16, tag="tr")
                        nc.tensor.transpose(
                            pt[:], src[:, 128 * c:128 * c + 128], id32b[:])
                        nc.vector.tensor_copy(dst[:, 32 * c:32 * c + 32], pt[:])

                def emit_logits(j):
                    base = 128 * j
                    mj = min(128, COLS - base)
                    sums = []
                    for gi_, (goff, gsz) in enumerate(groups):
                        pl = p2pl.tile([128, 1024], f32, tag="lg")
                        for soff in range(0, gsz, 512):
                            ssz = min(512, gsz - soff)
                            for c in range(4):
                                nc.tensor.matmul(
                                    pl[:mj, soff:soff + ssz],
                                    HT[:, c * COLS + base:
                                       c * COLS + base + mj],
                                    wout[:, c * V + goff + soff:
                                         c * V + goff + soff + ssz],
                                    start=(c == 0), stop=False)
                            nc.tensor.matmul(
                                pl[:mj, soff:soff + ssz],
                                zrepb[:, 0:mj],
                                wout[:, 4 * V + goff + soff:
                                     4 * V + goff + soff + ssz],
                                start=False, stop=False)
                            nc.tensor.matmul(
                                pl[:mj, soff:soff + ssz],
                                ones128b[0:1, 0:mj],
                                bout[0:1, goff + soff:goff + soff + ssz],
                                start=False, stop=True)
                        es = p2s.tile([128, 1024], bf16, tag="es")
                        sm = p2s.tile([128, 1], f32, tag=f"sm{gi_}")
                        nc.scalar.activation(es[:mj, 0:gsz], pl[:mj, 0:gsz],
                                             AF.Exp, accum_out=sm[:mj, :])
                        sums.append(sm)
                    a01 = p2s.tile([128, 1], f32, tag="a01")
                    nc.vector.tensor_add(a01[:mj], sums[0][:mj], sums[1][:mj])
                    a23 = p2s.tile([128, 1], f32, tag="a23")
                    nc.vector.tensor_add(a23[:mj], sums[2][:mj], sums[3][:mj])
                    a03 = p2s.tile([128, 1], f32, tag="a03")
                    nc.vector.tensor_add(a03[:mj], a01[:mj], a23[:mj])
                    se = p2s.tile([128, 1], f32, tag="se")
                    nc.vector.tensor_add(se[:mj], a03[:mj], sums[4][:mj])
                    nc.scalar.activation(lses[:mj, j:j + 1], se[:mj], AF.Ln)

                for t in range(NT):
                    jt, tl = t // 4, t % 4

                    # layer-0 gate matmuls, order f, i, cn, o
                    g0t = {}
                    for gate in (GF, GI, GC, GO):
                        off = 512 * gate
                        gp = p1g.tile([BL, 512], f32, tag="g")
                        for c in range(4):
                            nc.tensor.matmul(
                                gp[:, :], h0T[:, 32 * c:32 * c + 32],
                                w0h[:, c * G + off:c * G + off + 512],
                                start=(c == 0), stop=False)
                        nc.tensor.matmul(gp[:, :],
                                         selc[:, 32 * tl:32 * tl + 32],
                                         preS[:, jt * G + off:jt * G + off + 512],
                                         start=False, stop=True)
                        g0t[gate] = gp

                    # deferred layer-1 tail of the previous step
                    if pend is not None:
                        h1T, c1 = pend()
                        pend = None
                    if t >= 4 and t % 4 == 0:
                        emit_logits(t // 4 - 1)

                    # layer-0 gates
                    sf = p1e.tile([BL, D], bf16, tag="sf")
                    nc.scalar.activation(sf[:], g0t[GF][:], AF.Sigmoid)
                    si = p1e.tile([BL, D], bf16, tag="si")
                    nc.scalar.activation(si[:], g0t[GI][:], AF.Sigmoid)
                    cn = p1e.tile([BL, D], bf16, tag="cn")
                    nc.scalar.activation(cn[:], g0t[GC][:], AF.Tanh)
                    so = p1e.tile([BL, D], bf16, tag="so")
                    nc.scalar.activation(so[:], g0t[GO][:], AF.Sigmoid)
                    t1 = p1e.tile([BL, D], f32, tag="t1")
                    nc.vector.tensor_mul(t1[:], sf[:], c0)
                    t2 = p1e.tile([BL, D], f32, tag="t2")
                    nc.vector.tensor_mul(t2[:], si[:], cn[:])
                    c0n = state2.tile([BL, D], f32, tag="c0")
                    nc.vector.tensor_add(c0n[:], t1[:], t2[:])
                    th = p1e.tile([BL, D], bf16, tag="th")
                    nc.scalar.activation(th[:], c0n[:], AF.Tanh)
                    h0 = p1e.tile([BL, D], bf16, tag="h0")
                    nc.vector.tensor_mul(h0[:], so[:], th[:])
                    h0Tn = state2.tile([128, 128], bf16, tag="h0T")
                    transpose4(h0, h0Tn)

                    # layer-1 gate matmuls: h1/bias chunks first, h0 last
                    g1t = {}
                    for gate in (GF, GI, GC, GO):
                        off = 512 * gate
                        gp = p1g.tile([BL, 512], f32, tag="g")
                        for c in range(4):
                            nc.tensor.matmul(
                                gp[:, :], h1T[:, 32 * c:32 * c + 32],
                                w1[:, c * G + off:c * G + off + 512],
                                start=(c == 0), stop=False)
                        nc.tensor.matmul(gp[:, :], ones32[0:1, :],
                                         bg1[0:1, off:off + 512],
                                         start=False, stop=False)
                        for c in range(4):
                            nc.tensor.matmul(
                                gp[:, :], h0Tn[:, 32 * c:32 * c + 32],
                                w1[:, (4 + c) * G + off:
                                   (4 + c) * G + off + 512],
                                start=False, stop=(c == 3))
                        g1t[gate] = gp

                    sf1 = p1e.tile([BL, D], bf16, tag="sf")
                    nc.scalar.activation(sf1[:], g1t[GF][:], AF.Sigmoid)
                    si1 = p1e.tile([BL, D], bf16, tag="si")
                    nc.scalar.activation(si1[:], g1t[GI][:], AF.Sigmoid)
                    cn1 = p1e.tile([BL, D], bf16, tag="cn")
                    nc.scalar.activation(cn1[:], g1t[GC][:], AF.Tanh)
                    so1 = p1e.tile([BL, D], bf16, tag="so")
                    nc.scalar.activation(so1[:], g1t[GO][:], AF.Sigmoid)

                    def tail(t=t, sf1=sf1, si1=si1, cn1=cn1, so1=so1,
                             c1_old=c1, h0Tn=h0Tn):
                        u1 = p1e.tile([BL, D], f32, tag="t1")
                        nc.vector.tensor_mul(u1[:], sf1[:], c1_old)
                        u2 = p1e.tile([BL, D], f32, tag="t2")
                        nc.vector.tensor_mul(u2[:], si1[:], cn1[:])
                        c1n = state2.tile([BL, D], f32, tag="c1")
                        nc.vector.tensor_add(c1n[:], u1[:], u2[:])
                        th1 = p1e.tile([BL, D], bf16, tag="th")
                        nc.scalar.activation(th1[:], c1n[:], AF.Tanh)
                        h1 = p1e.tile([BL, D], bf16, tag="h0")
                        nc.vector.tensor_mul(h1[:], so1[:], th1[:])
                        h1Tn = state2.tile([128, 128], bf16, tag="h1T")
                        transpose4(h1, h1Tn)
                        for c in range(4):
                            nc.vector.tensor_add(
                                HT[:, c * COLS + BL * t:
                                   c * COLS + BL * t + BL],
                                h0Tn[:, 32 * c:32 * c + 32],
                                h1Tn[:, 32 * c:32 * c + 32])
                        return h1Tn, c1n[:]

                    pend = tail
                    h0T = h0Tn
                    c0 = c0n[:]
                    c1 = None  # produced by the deferred tail
                if pend is not None:
                    h1T, c1 = pend()
                    pend = None
                emit_logits(9)

            # ---------------- phase-2 tail: target dots, lp, output --------
            with tc.tile_pool(name="p2wb", bufs=2) as p2wb, \
                 tc.tile_pool(name="p2t", bufs=2) as p2t, \
                 tc.tile_pool(name="p2pd", bufs=2, space="PSUM") as p2pd:
                for j in range(NTILE):
                    base = 128 * j
                    mj = min(128, COLS - base)
                    wtac = p2wb.tile([128, 5 * 128], f32r, tag="wtac")
                    for c in range(5):
                        nc.sync.dma_start(
                            wtac[:, 128 * c:128 * c + mj],
                            wta_d[:, c * COLS + base:c * COLS + base + mj])
                    dps = p2pd.tile([128, 2], f32, tag="dot")
                    for c in range(5):
                        hx_c = (HT[:, c * COLS + base:c * COLS + base + mj]
                                if c < 4 else zrepb[:, 0:mj])
                        sc = p2t.tile([128, 128], f32r, tag="S")
                        nc.vector.tensor_mul(
                            sc[:, 0:mj], hx_c,
                            wtac[:, 128 * c:128 * c + mj])
                        nc.tensor.matmul(dps[:mj, 0:2], sc[:, 0:mj],
                                         onescol[:, :],
                                         start=(c == 0), stop=(c == 4))
                    lpt = p2t.tile([128, 1], f32, tag="lp")
                    nc.vector.tensor_sub(lpt[:mj], dps[:mj, 0:1],
                                         lses[:mj, j:j + 1])
                    nc.sync.dma_start(out_d[base:base + mj, :], lpt[:mj, :])
            p2w_cm.__exit__(None, None, None)
            p1w_cm.__exit__(None, None, None)

    nc.compile()
    return nc


def _prep_host(inputs):
    """Build per-core input maps from the full problem inputs."""
    z = np.asarray(inputs["z"], np.float32)
    x = np.asarray(inputs["x"])
    emb = np.asarray(inputs["emb"], np.float32)
    Wg0 = np.asarray(inputs["Wg0"], np.float32)
    bg0 = np.asarray(inputs["bg0"], np.float32)
    Wg1 = np.asarray(inputs["Wg1"], np.float32)
    bg1 = np.asarray(inputs["bg1"], np.float32)
    Wout = np.asarray(inputs["Wout"], np.float32)
    bout = np.asarray(inputs["bout"], np.float32)
    tw1 = np.asarray(inputs["tw1"], np.float32)
    tb1 = np.asarray(inputs["tb1"], np.float32)
    tw2 = np.asarray(inputs["tw2"], np.float32)
    tb2 = np.asarray(inputs["tb2"], np.float32)

    bf = ml_dtypes.bfloat16

    def chunked(a, nch):
        # [128*nch, N] -> [128, nch*N]
        n = a.shape[1]
        return np.ascontiguousarray(
            a.reshape(nch, 128, n).transpose(1, 0, 2).reshape(128, nch * n))

    shared = {
        "w0h": chunked(Wg0[:, :, 0:512].reshape(G, 512).T, 4).astype(bf),
        "w0e": chunked(Wg0[:, :, 512:1024].reshape(G, 512).T, 4).astype(bf),
        "w0z": np.ascontiguousarray(
            Wg0[:, :, 1024:1152].reshape(G, 128).T).astype(bf),
        "bg0r": bg0.reshape(1, G).astype(bf),
        "w1": chunked(Wg1.reshape(G, 1024).T, 8).astype(bf),
        "bg1r": bg1.reshape(1, G).astype(bf),
        "tw1T": np.concatenate([tw1[0].T, tw1[1].T], axis=1).astype(bf),
        "tb1r": tb1.reshape(1, 2 * G).astype(bf),
        "tw2T": np.concatenate(
            [chunked(tw2[0].T, 16), chunked(tw2[1].T, 16)], axis=1).astype(bf),
        "tb2r": tb2.reshape(1, 2 * 1024).astype(bf),
        "woutT": chunked(Wout.T[0:640], 5).astype(bf),
        "boutr": bout.reshape(1, V).astype(bf),
        "id32": np.eye(32, dtype=np.float32),
        "id32b": np.eye(32, dtype=bf),
        "selc": np.eye(128, dtype=bf),
        "ones32": np.ones((1, BL), bf),
        "ones128b": np.ones((1, 128), bf),
        "onescol": np.ones((128, 2), np.float32),
    }

    in_maps = []
    bout_extra = []
    for cidx in range(NC):
        bs = slice(BL * cidx, BL * cidx + BL)
        z_c = z[bs]                              # [32, 128]
        x_c = x[bs]                              # [32, 40]
        embx = emb[x_c]                          # [32, 40, 512]
        xn = x_c[:, 1:T]                         # [32, 39] targets
        wrows = Wout[xn]                         # [32, 39, 640]
        zT = np.ascontiguousarray(z_c.T)         # [128, 32]
        m = dict(shared)
        m["zT"] = zT.astype(bf)
        m["zrepb"] = np.tile(zT, (1, 4)).astype(bf)
        m["eT"] = np.ascontiguousarray(
            embx.transpose(2, 1, 0).reshape(4, 128, T * BL)
            .transpose(1, 0, 2).reshape(128, 4 * T * BL)).astype(bf)
        m["wtaT"] = np.ascontiguousarray(
            wrows.transpose(2, 1, 0).reshape(5, 128, COLS)
            .transpose(1, 0, 2).reshape(128, 5 * COLS)).astype(np.float32)
        in_maps.append(m)
        bout_extra.append(bout[xn].sum(axis=1))  # [32]
    return in_maps, bout_extra


def kernel(**inputs) -> np.ndarray:
    if "nc" not in _CACHE:
        _CACHE["nc"] = _build()
    nc = _CACHE["nc"]
    in_maps, bout_extra = _prep_host(inputs)
    res = bass_utils.run_bass_kernel_spmd(nc, in_maps, core_ids=list(range(NC)))
    out = np.zeros((B, 1), np.float32)
    for cidx in range(NC):
        lp = res.results[cidx]["out_lp"].reshape(NT, BL)   # [39, 32] t-major
        out[BL * cidx:BL * cidx + BL, 0] = lp.sum(axis=0) + bout_extra[cidx]
    return out


# revision 25
# speedup vs baseline: 1.0956x; 1.0956x over previous
"""Trainium2 Bass kernel for nn_Decoder: 2-layer LSTM decoder + log-softmax NLL.

Strategy: pure 8-way data parallel over batch (B=256 -> 32 rows/core), zero
collectives. Per core:
  pre:     batched precompute of the non-recurrent layer-0 gate contribution
           pre[t,b,:] = e @ W0e + z @ W0z + bg0 (full-width matmuls, PSUM ->
           DRAM scratch), re-injected per step with one identity matmul
  phase 0: transformh0 (z -> initial h/c per layer) on device
  phase 1: 39 recurrent LSTM steps; weights streamed through the PE as the
           moving operand (stationary = transposed activations, M=32);
           per-gate PSUM slices + per-gate activations for early release;
           layer-1 elementwise tail deferred past the next step's layer-0
           matmuls (software-pipelined emission)
  phase 2: vocab projection batched over (t, b) -> logsumexp via ACT exp with
           fused accum_out; target logit via elementwise mul + ones-matmul
           partition reduction against host-pregathered Wout rows.
Host does: embedding gather, weight transposes/reshapes, final sum over t.
LSTM matmul operands bf16 (fp32 PSUM accumulate); vocab matmuls float32r.
"""

import numpy as np
import ml_dtypes

import concourse.tile as tile
import concourse.mybir as mybir
from concourse import bacc
from concourse import bass_utils

B, T, V, D, Z = 256, 40, 5000, 512, 128
NC = 8
BL = B // NC            # 32 batch rows per core
NT = T - 1              # 39 recurrent steps / vocab rows per b
COLS = NT * BL          # 1248 (t, b) columns per core
G = 4 * D               # 2048 gate width
NTILE = (COLS + 127) // 128   # 10 vocab tiles (last has 96 cols)

bf16 = mybir.dt.bfloat16
f32 = mybir.dt.float32
f32r = mybir.dt.float32r
AF = mybir.ActivationFunctionType

# gate order in the fused weight layout: i, f, o, cn
GI, GF, GO, GC = 0, 1, 2, 3

_CACHE = {}


def _build():
    nc = bacc.Bacc("TRN2", target_bir_lowering=False, debug=False)

    def din(name, shape, dt):
        return nc.dram_tensor(name, shape, dt, kind="ExternalInput").ap()

    zT_d = din("zT", [128, BL], bf16)
    zrepb_d = din("zrepb", [128, 128], bf16)
    eT_d = din("eT", [128, 4 * T * BL], bf16)
    w0h_d = din("w0h", [128, 4 * G], bf16)
    w0e_d = din("w0e", [128, 4 * G], bf16)
    w0z_d = din("w0z", [128, G], bf16)
    bg0_d = din("bg0r", [1, G], bf16)
    w1_d = din("w1", [128, 8 * G], bf16)
    bg1_d = din("bg1r", [1, G], bf16)
    tw1_d = din("tw1T", [128, 2 * G], bf16)
    tb1_d = din("tb1r", [1, 2 * G], bf16)
    tw2_d = din("tw2T", [128, 2 * 16 * 1024], bf16)
    tb2_d = din("tb2r", [1, 2 * 1024], bf16)
    wout_d = din("woutT", [128, 5 * V], bf16)
    bout_d = din("boutr", [1, V], bf16)
    wta_d = din("wtaT", [128, 5 * COLS], f32r)
    id32_d = din("id32", [32, 32], f32)
    id32b_d = din("id32b", [32, 32], bf16)
    selc_d = din("selc", [128, 128], bf16)
    ones32_d = din("ones32", [1, BL], bf16)
    ones128b_d = din("ones128b", [1, 128], bf16)
    onescol_d = din("onescol", [128, 2], f32r)
    out_d = nc.dram_tensor("out_lp", [COLS, 1], f32, kind="ExternalOutput").ap()

    with tile.TileContext(nc) as tc:
        from contextlib import ExitStack
        with ExitStack() as ctx:
            const = ctx.enter_context(tc.tile_pool(name="const", bufs=1))
            state = ctx.enter_context(tc.tile_pool(name="state", bufs=1))
            state2 = ctx.enter_context(tc.tile_pool(name="state2", bufs=2))

            def cload(shape, dt, dram, tag):
                t = const.tile(shape, dt, tag=tag)
                nc.sync.dma_start(t[:], dram[:])
                return t

            zT = cload([128, BL], bf16, zT_d, "c_zT")
            zrepb = cload([128, 128], bf16, zrepb_d, "c_zrepb")
            id32 = cload([32, 32], f32, id32_d, "c_id32")
            id32b = cload([32, 32], bf16, id32b_d, "c_id32b")
            selc = cload([128, 128], bf16, selc_d, "c_selc")
            ones32 = cload([1, BL], bf16, ones32_d, "c_ones32")
            ones128b = cload([1, 128], bf16, ones128b_d, "c_ones128b")
            onescol = cload([128, 2], f32r, onescol_d, "c_onescol")
            bg0 = cload([1, G], bf16, bg0_d, "c_bg0")
            bg1 = cload([1, G], bf16, bg1_d, "c_bg1")

            HT = state.tile([128, 4 * COLS], bf16)
            preS = state.tile([128, NTILE * G], bf16, tag="preS")
            lses = state.tile([128, 16], f32, tag="lses")

            # recurrent-loop weights: pool reserved early so the DMAs can
            # stream during phase 0 / precompute without address conflicts
            p1w_cm = tc.tile_pool(name="p1w", bufs=1)
            p1w = p1w_cm.__enter__()

            # phase-0 weights (tw2 per-layer shared slot)
            p0w_cm = tc.tile_pool(name="p0w", bufs=1)
            p0w = p0w_cm.__enter__()
            tw1 = p0w.tile([128, 2 * G], bf16, tag="tw1")
            nc.sync.dma_start(tw1[:], tw1_d[:])
            # precompute inputs next in DMA priority order
            ppw_cm = tc.tile_pool(name="ppw", bufs=1)
            ppw = ppw_cm.__enter__()
            w0e = ppw.tile([128, 4 * G], bf16)
            nc.sync.dma_start(w0e[:], w0e_d[:])
            w0z = ppw.tile([128, G], bf16)
            nc.sync.dma_start(w0z[:], w0z_d[:])
            eT = ppw.tile([128, 4 * T * BL], bf16)
            nc.sync.dma_start(eT[:], eT_d[:])
            tw2a = p0w.tile([128, 16 * 1024], bf16, tag="tw2")
            nc.sync.dma_start(tw2a[:], tw2_d[:, 0:16384])
            w0h = p1w.tile([128, 4 * G], bf16)
            nc.sync.dma_start(w0h[:], w0h_d[:])
            w1 = p1w.tile([128, 8 * G], bf16)

            # ---------------- phase 0: transformh0 -------------------------
            # emitted before the precompute so the precompute matmuls fill the
            # PE gaps left by phase 0's transpose/activation chains
            c_prev = [None, None]
            hT_init = [None, None]
            with tc.tile_pool(name="p0s", bufs=1) as p0s, \
                 tc.tile_pool(name="p0pa", bufs=1, space="PSUM") as p0pa, \
                 tc.tile_pool(name="p0tr", bufs=2, space="PSUM") as p0tr, \
                 tc.tile_pool(name="ppp", bufs=2, space="PSUM") as ppp:
                def phase0_layer(layer):
                    if layer == 0:
                        tw2 = tw2a
                    else:
                        tw2 = p0w.tile([128, 16 * 1024], bf16, tag="tw2")
                        nc.sync.dma_start(
                            tw2[:], tw2_d[:, 16384:32768])
                    tb1 = p0w.tile([1, G], bf16, tag="tb1")
                    nc.sync.dma_start(tb1[:], tb1_d[0:1, layer * G:(layer + 1) * G])
                    tb2 = p0w.tile([1, 1024], bf16, tag="tb2")
                    nc.sync.dma_start(
                        tb2[:], tb2_d[0:1, layer * 1024:(layer + 1) * 1024])
                    pa = p0pa.tile([BL, G], f32, tag="pa")
                    for s in range(4):
                        ns = slice(512 * s, 512 * s + 512)
                        nc.tensor.matmul(pa[:, ns], zT[:, :],
                                         tw1[:, layer * G + 512 * s:
                                             layer * G + 512 * s + 512],
                                         start=True, stop=False)
                        nc.tensor.matmul(pa[:, ns], ones32[0:1, :],
                                         tb1[0:1, 512 * s:512 * s + 512],
                                         start=False, stop=True)
                    u = p0s.tile([BL, G], bf16, tag="u")
                    nc.scalar.activation(u[:], pa[:], AF.Relu)
                    uT = p0s.tile([128, 16 * 32], bf16, tag="uT")
                    for c in range(16):
                        pt = p0tr.tile([128, 32], bf16, tag="tr")
                        nc.tensor.transpose(pt[:], u[:, 128 * c:128 * c + 128],
                                            id32b[:])
                        nc.vector.tensor_copy(uT[:, 32 * c:32 * c + 32], pt[:])
                    pb = p0pa.tile([BL, G], f32, tag="pa")
                    for s in range(2):
                        ns = slice(512 * s, 512 * s + 512)
                        for c in range(16):
                            nc.tensor.matmul(
                                pb[:, ns], uT[:, 32 * c:32 * c + 32],
                                tw2[:, c * 1024 + 512 * s:
                                    c * 1024 + 512 * s + 512],
                                start=(c == 0), stop=False)
                        nc.tensor.matmul(pb[:, ns], ones32[0:1, :],
                                         tb2[0:1, 512 * s:512 * s + 512],
                                         start=False, stop=True)
                    v = state.tile([BL, 1024], f32, tag=f"v{layer}")
                    nc.scalar.activation(v[:], pb[:, 0:1024], AF.Tanh)
                    hT = state.tile([128, 128], bf16, tag=f"hTi{layer}")
                    for c in range(4):
                        pt = p0tr.tile([128, 32], f32, tag="tr")
                        nc.tensor.transpose(pt[:], v[:, 128 * c:128 * c + 128],
                                            id32[:])
                        nc.vector.tensor_copy(hT[:, 32 * c:32 * c + 32], pt[:])
                    hT_init[layer] = hT
                    c_prev[layer] = v[:, 512:1024]

                # ------- precompute pre[t,b,:] = eW0e + zW0z + bg0 ---------
                def pre_tile(j):
                    for q in range(4):
                        go = 512 * q
                        pp = ppp.tile([128, 512], f32, tag="pp")
                        for c in range(4):
                            nc.tensor.matmul(
                                pp[:, :],
                                eT[:, c * T * BL + 128 * j:
                                   c * T * BL + 128 * j + 128],
                                w0e[:, c * G + go:c * G + go + 512],
                                start=(c == 0), stop=False)
                        nc.tensor.matmul(pp[:, :], zrepb[:, :],
                                         w0z[:, go:go + 512],
                                         start=False, stop=False)
                        nc.tensor.matmul(pp[:, :], ones128b[0:1, :],
                                         bg0[0:1, go:go + 512],
                                         start=False, stop=True)
                        nc.scalar.copy(preS[:, j * G + go:j * G + go + 512],
                                       pp[:, :])

                phase0_layer(0)
                for j in range(8):
                    pre_tile(j)
                phase0_layer(1)
                nc.sync.dma_start(w1[:], w1_d[:])
                for j in range(8, NTILE):
                    pre_tile(j)

            ppw_cm.__exit__(None, None, None)
            p0w_cm.__exit__(None, None, None)

            # phase-2 vocab weights: load during phase 1 (DMA idle there)
            p2w_cm = tc.tile_pool(name="p2w", bufs=1)
            p2w = p2w_cm.__enter__()
            wout = p2w.tile([128, 5 * V], bf16)
            nc.gpsimd.dma_start(wout[:], wout_d[:])
            bout = p2w.tile([1, V], bf16)
            nc.gpsimd.dma_start(bout[:], bout_d[:])

            # ---------------- phase 1: 39 LSTM steps -----------------------
            # vocab logits tiles are interleaved into the loop as PE filler
            groups = [(0, 1024), (1024, 1024), (2048, 1024),
                      (3072, 1024), (4096, 904)]
            with tc.tile_pool(name="p1g", bufs=4, space="PSUM") as p1g, \
                 tc.tile_pool(name="p1tr", bufs=2, space="PSUM") as p1tr, \
                 tc.tile_pool(name="p1e", bufs=2) as p1e, \
                 tc.tile_pool(name="p2s", bufs=2) as p2s, \
                 tc.tile_pool(name="p2pl", bufs=1, space="PSUM") as p2pl:
                h0T, h1T = hT_init
                c0, c1 = c_prev
                pend = None   # deferred layer-1 tail of the previous step

                def transpose4(src, dst):
                    for c in range(4):
                        pt = p1tr.tile([128, 32], bf16, tag="tr")
                        nc.tensor.transpose(
                            pt[:], src[:, 128 * c:128 * c + 128], id32b[:])
                        nc.vector.tensor_copy(dst[:, 32 * c:32 * c + 32], pt[:])

                sums_by_tile = {}

                def emit_group(j, gi_):
                    base = 128 * j
                    mj = min(128, COLS - base)
                    goff, gsz = groups[gi_]
                    pl = p2pl.tile([128, 1024], f32, tag="lg")
                    for soff in range(0, gsz, 512):
                        ssz = min(512, gsz - soff)
                        for c in range(4):
                            nc.tensor.matmul(
                                pl[:mj, soff:soff + ssz],
                                HT[:, c * COLS + base:c * COLS + base + mj],
                                wout[:, c * V + goff + soff:
                                     c * V + goff + soff + ssz],
                                start=(c == 0), stop=False)
                        nc.tensor.matmul(
                            pl[:mj, soff:soff + ssz],
                            zrepb[:, 0:mj],
                            wout[:, 4 * V + goff + soff:
                                 4 * V + goff + soff + ssz],
                            start=False, stop=False)
                        nc.tensor.matmul(
                            pl[:mj, soff:soff + ssz],
                            ones128b[0:1, 0:mj],
                            bout[0:1, goff + soff:goff + soff + ssz],
                            start=False, stop=True)
                    es = p2s.tile([128, 1024], bf16, tag="es")
                    sm = p2s.tile([128, 1], f32, tag=f"sm{gi_}")
                    nc.scalar.activation(es[:mj, 0:gsz], pl[:mj, 0:gsz],
                                         AF.Exp, accum_out=sm[:mj, :])
                    sums_by_tile.setdefault(j, []).append(sm)

                def finalize_tile(j):
                    mj = min(128, COLS - 128 * j)
                    sums = sums_by_tile.pop(j)
                    a01 = p2s.tile([128, 1], f32, tag="a01")
                    nc.vector.tensor_add(a01[:mj], sums[0][:mj], sums[1][:mj])
                    a23 = p2s.tile([128, 1], f32, tag="a23")
                    nc.vector.tensor_add(a23[:mj], sums[2][:mj], sums[3][:mj])
                    a03 = p2s.tile([128, 1], f32, tag="a03")
                    nc.vector.tensor_add(a03[:mj], a01[:mj], a23[:mj])
                    se = p2s.tile([128, 1], f32, tag="se")
                    nc.vector.tensor_add(se[:mj], a03[:mj], sums[4][:mj])
                    nc.scalar.activation(lses[:mj, j:j + 1], se[:mj], AF.Ln)

                vwork = []
                vpushed = 0

                def vocab_pump(t_done, n):
                    # tiles whose HT cols are complete: 4j+3 <= t_done
                    nonlocal vpushed
                    while vpushed < NTILE and min(4 * vpushed + 3, NT - 1) <= t_done:
                        j = vpushed
                        for gi_ in range(5):
                            vwork.append(("g", j, gi_))
                        vwork.append(("f", j, 0))
                        vpushed += 1
                    for _ in range(n):
                        if not vwork:
                            return
                        kind, j, gi_ = vwork.pop(0)
                        if kind == "g":
                            emit_group(j, gi_)
                        else:
                            finalize_tile(j)

                for t in range(NT):
                    jt, tl = t // 4, t % 4

                    # layer-0 gate matmuls, order f, i, cn, o
                    g0t = {}
                    for gate in (GF, GI, GC, GO):
                        off = 512 * gate
                        gp = p1g.tile([BL, 512], f32, tag="g")
                        for c in range(4):
                            nc.tensor.matmul(
                                gp[:, :], h0T[:, 32 * c:32 * c + 32],
                                w0h[:, c * G + off:c * G + off + 512],
                                start=(c == 0), stop=False)
                        nc.tensor.matmul(gp[:, :],
                                         selc[:, 32 * tl:32 * tl + 32],
                                         preS[:, jt * G + off:jt * G + off + 512],
                                         start=False, stop=True)
                        g0t[gate] = gp

                    # deferred layer-1 tail of the previous step
                    if pend is not None:
                        h1T, c1 = pend()
                        pend = None
                    vocab_pump(t - 1, 2 if len(vwork) > 6 else 1)

                    # layer-0 gates
                    sf = p1e.tile([BL, D], bf16, tag="sf")
                    nc.scalar.activation(sf[:], g0t[GF][:], AF.Sigmoid)
                    si = p1e.tile([BL, D], bf16, tag="si")
                    nc.scalar.activation(si[:], g0t[GI][:], AF.Sigmoid)
                    cn = p1e.tile([BL, D], bf16, tag="cn")
                    nc.scalar.activation(cn[:], g0t[GC][:], AF.Tanh)
                    so = p1e.tile([BL, D], bf16, tag="so")
                    nc.scalar.activation(so[:], g0t[GO][:], AF.Sigmoid)
                    t1 = p1e.tile([BL, D], f32, tag="t1")
                    nc.vector.tensor_mul(t1[:], sf[:], c0)
                    t2 = p1e.tile([BL, D], f32, tag="t2")
                    nc.vector.tensor_mul(t2[:], si[:], cn[:])
                    c0n = state2.tile([BL, D], f32, tag="c0")
                    nc.vector.tensor_add(c0n[:], t1[:], t2[:])
                    th = p1e.tile([BL, D], bf16, tag="th")
                    nc.scalar.activation(th[:], c0n[:], AF.Tanh)
                    h0 = p1e.tile([BL, D], bf16, tag="h0")
                    nc.vector.tensor_mul(h0[:], so[:], th[:])
                    h0Tn = state2.tile([128, 128], bf16, tag="h0T")
                    transpose4(h0, h0Tn)

                    # layer-1 gate matmuls: h1/bias chunks first, h0 last
                    g1t = {}
                    for gate in (GF, GI, GC, GO):
                        off = 512 * gate
                        gp = p1g.tile([BL, 512], f32, tag="g")
                        for c in range(4):
                            nc.tensor.matmul(
                                gp[:, :], h1T[:, 32 * c:32 * c + 32],
                                w1[:, c * G + off:c * G + off + 512],
                                start=(c == 0), stop=False)
                        nc.tensor.matmul(gp[:, :], ones32[0:1, :],
                                         bg1[0:1, off:off + 512],
                                         start=False, stop=False)
                        for c in range(4):
                            nc.tensor.matmul(
                                gp[:, :], h0Tn[:, 32 * c:32 * c + 32],
                                w1[:, (4 + c) * G + off:
                                   (4 + c) * G + off + 512],
                                start=False, stop=(c == 3))
                        g1t[gate] = gp

                    sf1 = p1e.tile([BL, D], bf16, tag="sf")
                    nc.scalar.activation(sf1[:], g1t[GF][:], AF.Sigmoid)
                    si1 = p1e.tile([BL, D], bf16, tag="si")
                    nc.scalar.activation(si1[:], g1t[GI][:], AF.Sigmoid)
                    cn1 = p1e.tile([BL, D], bf16, tag="cn")
                    nc.scalar.activation(cn1[:], g1t[GC][:], AF.Tanh)
                    so1 = p1e.tile([BL, D], bf16, tag="so")
                    nc.scalar.activation(so1[:], g1t[GO][:], AF.Sigmoid)

                    def tail(t=t, sf1=sf1, si1=si1, cn1=cn1, so1=so1,
                             c1_old=c1, h0Tn=h0Tn):
                        u1 = p1e.tile([BL, D], f32, tag="t1")
                        nc.vector.tensor_mul(u1[:], sf1[:], c1_old)
                        u2 = p1e.tile([BL, D], f32, tag="t2")
                        nc.vector.tensor_mul(u2[:], si1[:], cn1[:])
                        c1n = state2.tile([BL, D], f32, tag="c1")
                        nc.vector.tensor_add(c1n[:], u1[:], u2[:])
                        th1 = p1e.tile([BL, D], bf16, tag="th")
                        nc.scalar.activation(th1[:], c1n[:], AF.Tanh)
                        h1 = p1e.tile([BL, D], bf16, tag="h0")
                        nc.vector.tensor_mul(h1[:], so1[:], th1[:])
                        h1Tn = state2.tile([128, 128], bf16, tag="h1T")
                        transpose4(h1, h1Tn)
                        for c in range(4):
                            nc.vector.tensor_add(
                                HT[:, c * COLS + BL * t:
                                   c * COLS + BL * t + BL],
                                h0Tn[:, 32 * c:32 * c + 32],
                                h1Tn[:, 32 * c:32 * c + 32])
                        return h1Tn, c1n[:]

                    pend = tail
                    h0T = h0Tn
                    c0 = c0n[:]
                    c1 = None  # produced by the deferred tail
                if pend is not None:
                    h1T, c1 = pend()
                    pend = None
                vocab_pump(NT - 1, len(vwork) + 12)

            # ---------------- phase-2 tail: target dots, lp, output --------
            with tc.tile_pool(name="p2wb", bufs=2) as p2wb, \
                 tc.tile_pool(name="p2t", bufs=2) as p2t, \
                 tc.tile_pool(name="p2pd", bufs=2, space="PSUM") as p2pd:
                for j in range(NTILE):
                    base = 128 * j
                    mj = min(128, COLS - base)
                    wtac = p2wb.tile([128, 5 * 128], f32r, tag="wtac")
                    for c in range(5):
                        nc.sync.dma_start(
                            wtac[:, 128 * c:128 * c + mj],
                            wta_d[:, c * COLS + base:c * COLS + base + mj])
                    dps = p2pd.tile([128, 2], f32, tag="dot")
                    for c in range(5):
                        hx_c = (HT[:, c * COLS + base:c * COLS + base + mj]
                                if c < 4 else zrepb[:, 0:mj])
                        sc = p2t.tile([128, 128], f32r, tag="S")
                        nc.vector.tensor_mul(
                            sc[:, 0:mj], hx_c,
                            wtac[:, 128 * c:128 * c + mj])
                        nc.tensor.matmul(dps[:mj, 0:2], sc[:, 0:mj],
                                         onescol[:, :],
                                         start=(c == 0), stop=(c == 4))
                    lpt = p2t.tile([128, 1], f32, tag="lp")
                    nc.vector.tensor_sub(lpt[:mj], dps[:mj, 0:1],
                                         lses[:mj, j:j + 1])
                    nc.sync.dma_start(out_d[base:base + mj, :], lpt[:mj, :])
            p2w_cm.__exit__(None, None, None)
            p1w_cm.__exit__(None, None, None)

    nc.compile()
    return nc


def _prep_host(inputs):
    """Build per-core input maps from the full problem inputs."""
    z = np.asarray(inputs["z"], np.float32)
    x = np.asarray(inputs["x"])
    emb = np.asarray(inputs["emb"], np.float32)
    Wg0 = np.asarray(inputs["Wg0"], np.float32)
    bg0 = np.asarray(inputs["bg0"], np.float32)
    Wg1 = np.asarray(inputs["Wg1"], np.float32)
    bg1 = np.asarray(inputs["bg1"], np.float32)
    Wout = np.asarray(inputs["Wout"], np.float32)
    bout = np.asarray(inputs["bout"], np.float32)
    tw1 = np.asarray(inputs["tw1"], np.float32)
    tb1 = np.asarray(inputs["tb1"], np.float32)
    tw2 = np.asarray(inputs["tw2"], np.float32)
    tb2 = np.asarray(inputs["tb2"], np.float32)

    bf = ml_dtypes.bfloat16

    def chunked(a, nch):
        # [128*nch, N] -> [128, nch*N]
        n = a.shape[1]
        return np.ascontiguousarray(
            a.reshape(nch, 128, n).transpose(1, 0, 2).reshape(128, nch * n))

    shared = {
        "w0h": chunked(Wg0[:, :, 0:512].reshape(G, 512).T, 4).astype(bf),
        "w0e": chunked(Wg0[:, :, 512:1024].reshape(G, 512).T, 4).astype(bf),
        "w0z": np.ascontiguousarray(
            Wg0[:, :, 1024:1152].reshape(G, 128).T).astype(bf),
        "bg0r": bg0.reshape(1, G).astype(bf),
        "w1": chunked(Wg1.reshape(G, 1024).T, 8).astype(bf),
        "bg1r": bg1.reshape(1, G).astype(bf),
        "tw1T": np.concatenate([tw1[0].T, tw1[1].T], axis=1).astype(bf),
        "tb1r": tb1.reshape(1, 2 * G).astype(bf),
        "tw2T": np.concatenate(
            [chunked(tw2[0].T, 16), chunked(tw2[1].T, 16)], axis=1).astype(bf),
        "tb2r": tb2.reshape(1, 2 * 1024).astype(bf),
        "woutT": chunked(Wout.T[0:640], 5).astype(bf),
        "boutr": bout.reshape(1, V).astype(bf),
        "id32": np.eye(32, dtype=np.float32),
        "id32b": np.eye(32, dtype=bf),
        "selc": np.eye(128, dtype=bf),
        "ones32": np.ones((1, BL), bf),
        "ones128b": np.ones((1, 128), bf),
        "onescol": np.ones((128, 2), np.float32),
    }

    in_maps = []
    bout_extra = []
    for cidx in range(NC):
        bs = slice(BL * cidx, BL * cidx + BL)
        z_c = z[bs]                              # [32, 128]
        x_c = x[bs]                              # [32, 40]
        embx = emb[x_c]                          # [32, 40, 512]
        xn = x_c[:, 1:T]                         # [32, 39] targets
        wrows = Wout[xn]                         # [32, 39, 640]
        zT = np.ascontiguousarray(z_c.T)         # [128, 32]
        m = dict(shared)
        m["zT"] = zT.astype(bf)
        m["zrepb"] = np.tile(zT, (1, 4)).astype(bf)
        m["eT"] = np.ascontiguousarray(
            embx.transpose(2, 1, 0).reshape(4, 128, T * BL)
            .transpose(1, 0, 2).reshape(128, 4 * T * BL)).astype(bf)
        m["wtaT"] = np.ascontiguousarray(
            wrows.transpose(2, 1, 0).reshape(5, 128, COLS)
            .transpose(1, 0, 2).reshape(128, 5 * COLS)).astype(np.float32)
        in_maps.append(m)
        bout_extra.append(bout[xn].sum(axis=1))  # [32]
    return in_maps, bout_extra


def kernel(**inputs) -> np.ndarray:
    if "nc" not in _CACHE:
        _CACHE["nc"] = _build()
    nc = _CACHE["nc"]
    in_maps, bout_extra = _prep_host(inputs)
    res = bass_utils.run_bass_kernel_spmd(nc, in_maps, core_ids=list(range(NC)))
    out = np.zeros((B, 1), np.float32)
    for cidx in range(NC):
        lp = res.results[cidx]["out_lp"].reshape(NT, BL)   # [39, 32] t-major
        out[BL * cidx:BL * cidx + BL, 0] = lp.sum(axis=0) + bout_extra[cidx]
    return out
